# revision 1
# baseline (speedup 1.0000x reference)
"""Trainium2 Bass kernel for CustomTemporalAttention.

B=8, T=1024, E=1024, H=16, D=64. Sharding: pure batch data-parallel across the
8 NeuronCores (core b computes batch element b end-to-end; weights and the tiny
bias table are replicated). No collectives.

Per-core math (torch Linear convention x @ W.T + b):
  qT = Wq @ query[b].T  (stored transposed: [E, T], channel-major)
  kT likewise; v in [T, E] layout augmented with a ones column per head.
  Per head h: S^T[tk, tq] = sum_d kT[d,tk] qT[d,tq]
  P^T = exp(0.125 * (S^T + 8*biasT)) via DVE add + ACT exp(scale=0.125)
  [num; den] = [v_h | 1].T @ P^T  accumulated over tk chunks (PSUM [65, 512])
  O^T_h = num / den ; y = O @ Wo.T + bo.

Matmul operands are bf16 (fp32 PSUM accumulate): full-chain numpy model gives
5.1e-3 max rel err vs the fp32 reference. bf16 restores fast-weight-load and
LDWEIGHTS/ MATMUL overlap that fp32/f32r modes forfeit.

Temporal bias: bias(q,k) = lerp(table[q - k + T-1]) with a global fractional
shift u = tanh(offset)/2.  blend[r] = a*tabp[r] + b*tabp[r+1] + c*tabp[r+2]
with a=relu(-u), b=1-|u|, c=relu(u) and tabp edge-padded — exact including the
clipped endpoints. Toeplitz tiles blend[C' - i + j] are materialized per head
as BSp[i, y] = rblend[y + i] (one overlapping-window DMA from DRAM scratch)
and read back with reversed free-dim slices (both patterns HW-verified).
"""

import sys

sys.path.insert(0, "/opt/trn_rl_repo")

import ml_dtypes
import numpy as np

import concourse.bass as bass
import concourse.mybir as mybir
import concourse.tile as tile
from concourse.bass_utils import run_bass_kernel_spmd

F32 = mybir.dt.float32
BF16 = mybir.dt.bfloat16
AF = mybir.ActivationFunctionType
ALU = mybir.AluOpType

B, T, E, H = 8, 1024, 1024, 16
D = E // H  # 64
TQ = 512
W_BSP = 1920


def _split_multi_waits(nc):
    """This walrus build accepts at most one sync-wait per instruction; hoist
    extras onto same-engine NoOp carriers placed immediately before."""
    n = 0
    for f in nc.m.functions:
        for blk in f.blocks:
            out = []
            for inst in blk.instructions:
                si = inst.sync_info
                waits = list(si.on_wait) if si and si.on_wait else []
                if len(waits) > 1:
                    for w in waits[:-1]:
                        n += 1
                        nop = mybir.InstNoOp(name=f"{inst.name}-ws{n}", ins=[], outs=[])
                        nop.engine = inst.engine
                        nop.sync_info = mybir.SyncInfo(on_wait=[w], on_update=[])
                        out.append(nop)
                    inst.sync_info = mybir.SyncInfo(
                        on_wait=[waits[-1]], on_update=list(si.on_update or [])
                    )
                out.append(inst)
            blk.instructions = out
    return n


def _craft(ap, dims, offset=None):
    c = ap.copy()
    c.ap = ap.ap.__class__(dims)
    if offset is not None:
        c.offset = offset
    return c


def _build():
    nc = bass.Bass()

    xqT = nc.declare_dram_parameter("xqT", [E, T], BF16, isOutput=False)
    xkT = nc.declare_dram_parameter("xkT", [E, T], BF16, isOutput=False)
    xvT = nc.declare_dram_parameter("xvT", [E, T], BF16, isOutput=False)
    wqT = nc.declare_dram_parameter("wqT", [E, E], BF16, isOutput=False)
    wkT = nc.declare_dram_parameter("wkT", [E, E], BF16, isOutput=False)
    wvT = nc.declare_dram_parameter("wvT", [E, E], BF16, isOutput=False)
    woT = nc.declare_dram_parameter("woT", [E, E], BF16, isOutput=False)
    bq2 = nc.declare_dram_parameter("bq2", [128, 8], F32, isOutput=False)
    bk2 = nc.declare_dram_parameter("bk2", [128, 8], F32, isOutput=False)
    bv1 = nc.declare_dram_parameter("bv1", [E], F32, isOutput=False)
    bo1 = nc.declare_dram_parameter("bo1", [E], F32, isOutput=False)
    rtabp = nc.declare_dram_parameter("rtabp", [H, 2 * T + 1], F32, isOutput=False)
    offs = nc.declare_dram_parameter("offs", [1], F32, isOutput=False)
    y_out = nc.declare_dram_parameter("y", [T, E], F32, isOutput=True)

    with tile.TileContext(nc) as tc:
        with (
            tc.tile_pool(name="persist", bufs=1) as persist,
            tc.tile_pool(name="small", bufs=1) as small,
            tc.tile_pool(name="dram", bufs=1, space="DRAM") as drp,
        ):
            qT = persist.tile([128, 8, T], BF16, tag="qT")
            kT = persist.tile([128, 8, T], BF16, tag="kT")
            vp = persist.tile([128, 8, H, D + 1], BF16, tag="vp")
            oT = persist.tile([128, 8, T], BF16, tag="oT")
            bvrep = persist.tile([128, E], F32, tag="bvrep")
            borep = persist.tile([128, E], F32, tag="borep")
            bqs = small.tile([128, 8], F32, tag="bqs")
            bks = small.tile([128, 8], F32, tag="bks")

            nc.sync.dma_start(out=bqs[:], in_=bq2[:])
            nc.sync.dma_start(out=bks[:], in_=bk2[:])
            nc.sync.dma_start(out=bvrep[:], in_=_craft(bv1[:], [[0, 128], [1, E]], 0))
            nc.sync.dma_start(out=borep[:], in_=_craft(bo1[:], [[0, 128], [1, E]], 0))

            # ---- phase 0: blended relative-position table ----
            p0ctx = tc.tile_pool(name="p0", bufs=1)
            p0 = p0ctx.__enter__()
            tab = p0.tile([H, 2 * T + 1], F32, tag="tab")
            nc.sync.dma_start(out=tab[:], in_=rtabp[:])
            off_sb = p0.tile([1, 1], F32, tag="off")
            nc.sync.dma_start(out=off_sb[:], in_=offs[None, :])
            th = p0.tile([1, 1], F32, tag="th")
            nc.scalar.activation(th[:], off_sb[:], AF.Tanh)
            w8 = p0.tile([1, 1], F32, tag="w8")
            nc.vector.tensor_scalar_mul(w8[:], th[:], 4.0)  # 8*u = 4*tanh
            abc = p0.tile([1, 3], F32, tag="abc")
            nc.vector.tensor_scalar(abc[:, 0:1], w8[:], -1.0, 0.0, ALU.mult, ALU.max)
            nc.vector.tensor_scalar(abc[:, 2:3], w8[:], 1.0, 0.0, ALU.mult, ALU.max)
            tsum = p0.tile([1, 1], F32, tag="tsum")
            nc.vector.tensor_tensor(tsum[:], abc[:, 0:1], abc[:, 2:3], ALU.add)
            nc.vector.tensor_scalar(abc[:, 1:2], tsum[:], -1.0, 8.0, ALU.mult, ALU.add)
            abc_dram = drp.tile([3], F32, tag="abc_dram")
            nc.gpsimd.dma_start(out=abc_dram[None, :], in_=abc[:])
            abc16 = p0.tile([H, 3], F32, tag="abc16")
            nc.gpsimd.dma_start(out=abc16[:], in_=_craft(abc_dram[:], [[0, H], [1, 3]], 0))

            nblend = 2 * T - 1
            rb = p0.tile([H, nblend], F32, tag="rb")
            rb_t = p0.tile([H, nblend], F32, tag="rb_t")
            nc.vector.tensor_scalar(rb[:], tab[:, 2 : 2 + nblend], abc16[:, 0:1], None, ALU.mult)
            nc.vector.tensor_scalar(rb_t[:], tab[:, 1 : 1 + nblend], abc16[:, 1:2], None, ALU.mult)
            nc.vector.tensor_tensor(rb[:], rb[:], rb_t[:], ALU.add)
            nc.vector.tensor_scalar(rb_t[:], tab[:, 0:nblend], abc16[:, 2:3], None, ALU.mult)
            nc.vector.tensor_tensor(rb[:], rb[:], rb_t[:], ALU.add)
            # erb = exp(bias) of the blended table (bf16): per-head exp(b)
            # slabs are then pure overlapping-window DMA loads, no ACT work
            erb = p0.tile([H, nblend], BF16, tag="erb")
            nc.scalar.activation(erb[:], rb[:], AF.Exp, scale=0.125)
            erb_dram = drp.tile([H, nblend], BF16, tag="erb_dram")
            nc.gpsimd.dma_start(out=erb_dram[:], in_=erb[:])
            p0ctx.__exit__(None, None, None)

            # ---- phase 1: projections ----
            with (
                tc.tile_pool(name="xt", bufs=2) as xtp,
                tc.tile_pool(name="wt", bufs=10) as wtp,
                tc.tile_pool(name="wtv", bufs=1) as wtvp,
                tc.tile_pool(name="pps", bufs=4, space="PSUM") as pps,
            ):
                for name, x_in, w_in, dst, bias_sb in (
                    ("q", xqT, wqT, qT, bqs),
                    ("k", xkT, wkT, kT, bks),
                ):
                    xt = []
                    for eo in range(8):
                        for tq in range(2):
                            t_ = xtp.tile([128, TQ], BF16, tag=f"xt{eo}_{tq}")
                            nc.sync.dma_start(
                                out=t_[:],
                                in_=x_in[128 * eo : 128 * eo + 128, TQ * tq : TQ * tq + TQ],
                            )
                            xt.append(t_)
                    for fo in range(8):
                        ps = [pps.tile([128, TQ], F32, tag="pps", name=f"pp{fo}_{i}") for i in range(2)]
                        for eo in range(8):
                            wt_ = wtp.tile([128, 128], BF16, tag="wt")
                            nc.sync.dma_start(
                                out=wt_[:],
                                in_=w_in[128 * eo : 128 * eo + 128, 128 * fo : 128 * fo + 128],
                            )
                            for tq in range(2):
                                nc.tensor.matmul(
                                    ps[tq][:],
                                    wt_[:],
                                    xt[2 * eo + tq][:],
                                    start=(eo == 0),
                                    stop=(eo == 7),
                                )
                        for tq in range(2):
                            nc.vector.tensor_scalar(
                                dst[:, fo, TQ * tq : TQ * tq + TQ],
                                ps[tq][:],
                                1.0,
                                bias_sb[:, fo : fo + 1],
                                ALU.mult,
                                ALU.add,
                            )

                xt = []
                for eo in range(8):
                    for to2 in range(2):
                        t_ = xtp.tile([128, TQ], BF16, tag=f"xt{eo}_{to2}")
                        nc.sync.dma_start(
                            out=t_[:],
                            in_=xvT[128 * eo : 128 * eo + 128, TQ * to2 : TQ * to2 + TQ],
                        )
                        xt.append(t_)
                for fv in range(2):
                    wts = []
                    for eo in range(8):
                        wt_ = wtvp.tile([128, TQ], BF16, tag=f"wtv{eo}")
                        nc.sync.dma_start(
                            out=wt_[:],
                            in_=wvT[128 * eo : 128 * eo + 128, TQ * fv : TQ * fv + TQ],
                        )
                        wts.append(wt_)
                    for to in range(8):
                        to2, toi = divmod(to, 4)
                        ps = pps.tile([128, TQ], F32, tag="pps")
                        for eo in range(8):
                            nc.tensor.matmul(
                                ps[:],
                                xt[2 * eo + to2][:, 128 * toi : 128 * toi + 128],
                                wts[eo][:],
                                start=(eo == 0),
                                stop=(eo == 7),
                            )
                        nc.vector.tensor_tensor(
                            vp[:, to, 8 * fv : 8 * fv + 8, 0:D],
                            ps[:].rearrange("p (h d) -> p h d", d=D),
                            bvrep[:, TQ * fv : TQ * fv + TQ].rearrange(
                                "p (h d) -> p h d", d=D
                            ),
                            ALU.add,
                        )
                nc.vector.memset(vp[:, :, :, D : D + 1], 1.0)

            # ---- phase 2: attention ----
            # exp(0.125*(S + 8b)) = exp(0.125*S) * exp(b): the Toeplitz bias is
            # applied multiplicatively with a per-head exp(b) slab (bf16, DVE
            # 4x mode) instead of an fp32 PSUM add, and PV matmuls are emitted
            # as a block after the S block so the PE stream never stalls on
            # the exp chain.
            with (
                tc.tile_pool(name="eb", bufs=2) as ebp,
                tc.tile_pool(name="pt", bufs=4) as ptp,
                tc.tile_pool(name="pt0", bufs=3) as pt0p,
                tc.tile_pool(name="sm", bufs=6) as smp,
                tc.tile_pool(name="onum", bufs=6) as onp,
                tc.tile_pool(name="sps", bufs=2, space="PSUM") as sps,
                tc.tile_pool(name="ops", bufs=4, space="PSUM") as ops,
                tc.tile_pool(name="dr2", bufs=6, space="DRAM") as drp2,
            ):
                ebs = {}
                pend = {}

                def emit_eb(hh):
                    eb_ = ebp.tile([128, W_BSP], BF16, tag="eb", name=f"eb{hh}")
                    nc.sync.dma_start(
                        out=eb_[:],
                        in_=_craft(erb_dram[:], [[1, 128], [1, W_BSP]], hh * nblend),
                    )
                    ebs[hh] = eb_

                def _norm_stage_a(hh):
                    st = pend[hh]
                    for tq in range(2):
                        opsum_ = st["opsum"][tq]
                        den = smp.tile([1, TQ], F32, tag="den", name=f"den{hh}_{tq}")
                        nc.vector.tensor_copy(out=den[:], in_=opsum_[D : D + 1, :])
                        onum = onp.tile([64, TQ], F32, tag="onum", name=f"on{hh}_{tq}")
                        nc.vector.tensor_copy(out=onum[:], in_=opsum_[0:D, :])
                        den_dram = drp2.tile([TQ], F32, tag="dend", name=f"dd{hh}_{tq}")
                        nc.gpsimd.dma_start(out=den_dram[None, :], in_=den[:])
                        den4 = smp.tile([128, 4], F32, tag="den4", name=f"d4{hh}_{tq}")
                        nc.gpsimd.dma_start(
                            out=den4[:], in_=den_dram.rearrange("(f p) -> p f", p=128)
                        )
                        st["den"].append(den)
                        st["onum"].append(onum)
                        st["den4"].append(den4)

                def _norm_stage_b(hh):
                    st = pend[hh]
                    for tq in range(2):
                        rec4 = smp.tile([128, 4], F32, tag="rec4", name=f"r4{hh}_{tq}")
                        nc.vector.reciprocal(rec4[:], st["den4"][tq][:])
                        rec_dram = drp2.tile([TQ], F32, tag="recd", name=f"rd{hh}_{tq}")
                        nc.gpsimd.dma_start(
                            out=rec_dram.rearrange("(f p) -> p f", p=128), in_=rec4[:]
                        )
                        rep = onp.tile([64, TQ], F32, tag="rep", name=f"rp{hh}_{tq}")
                        nc.gpsimd.dma_start(
                            out=rep[:], in_=_craft(rec_dram[:], [[0, 64], [1, TQ]], 0)
                        )
                        st["rec4"].append(rec4)
                        st["rep"].append(rep)

                def _norm_stage_c(hh):
                    st = pend.pop(hh)
                    po_, hp0_ = st["po"], st["hp0"]
                    for tq in range(2):
                        if hp0_ == 0:
                            nc.gpsimd.tensor_tensor(
                                oT[0:64, po_, TQ * tq : TQ * tq + TQ],
                                st["onum"][tq][:],
                                st["rep"][tq][:],
                                ALU.mult,
                            )
                        else:
                            onrm = onp.tile([64, TQ], BF16, tag="onrm", name=f"om{hh}_{tq}")
                            nc.gpsimd.tensor_tensor(
                                onrm[:], st["onum"][tq][:], st["rep"][tq][:], ALU.mult
                            )
                            nc.gpsimd.dma_start(
                                out=oT[64:128, po_, TQ * tq : TQ * tq + TQ], in_=onrm[:]
                            )

                emit_eb(0)
                for h in range(H):
                    hp0 = 64 * (h % 2)
                    po = h // 2
                    if h + 1 < H:
                        emit_eb(h + 1)
                    eb = ebs.pop(h)
                    opsum = [
                        ops.tile([D + 1, TQ], F32, tag="ops", name=f"op{h}_{i}")
                        for i in range(2)
                    ]
                    pts = {}

                    def emit_pv(cc):
                        pt_ = pts.pop(cc)
                        for tq in range(2):
                            nc.tensor.matmul(
                                opsum[tq][:],
                                vp[:, cc, h, :],
                                pt_[:, TQ * tq : TQ * tq + TQ],
                                start=(cc == 0),
                                stop=(cc == 7),
                            )

                    for c in range(8):
                        # both tq halves share one 2-bank PSUM tile so the exp
                        # and bias-multiply run as single [128,1024] ops
                        spsum = sps.tile([128, 2 * TQ], F32, tag="sps", name=f"sp{h}_{c}")
                        for tq in range(2):
                            nc.tensor.matmul(
                                spsum[:, TQ * tq : TQ * tq + TQ],
                                kT[hp0 : hp0 + 64, po, 128 * c : 128 * c + 128],
                                qT[hp0 : hp0 + 64, po, TQ * tq : TQ * tq + TQ],
                                start=True,
                                stop=True,
                            )
                        pt0 = pt0p.tile([128, 2 * TQ], BF16, tag="pt0")
                        nc.scalar.activation(pt0[:], spsum[:], AF.Exp, scale=0.125)
                        s0 = 1023 + 128 * c
                        pt = ptp.tile([128, 2 * TQ], BF16, tag="pt")
                        nc.vector.tensor_tensor(
                            pt[:],
                            pt0[:],
                            eb[:, s0 - (2 * TQ - 1) : s0 + 1][:, ::-1],
                            ALU.mult,
                        )
                        pts[c] = pt
                        if c >= 1:
                            emit_pv(c - 1)
                    emit_pv(7)
                    # normalize runs 1-3 heads deferred so nothing in any
                    # engine FIFO waits on a fresh PV-block or DMA roundtrip
                    pend[h] = {"po": po, "hp0": hp0, "opsum": opsum, "den": [],
                               "onum": [], "den4": [], "rec4": [], "rep": []}
                    if h - 1 in pend:
                        _norm_stage_a(h - 1)
                    if h - 2 in pend:
                        _norm_stage_b(h - 2)
                    if h - 3 in pend:
                        _norm_stage_c(h - 3)
                _norm_stage_a(H - 1)
                _norm_stage_b(H - 2)
                _norm_stage_c(H - 3)
                _norm_stage_b(H - 1)
                _norm_stage_c(H - 2)
                _norm_stage_c(H - 1)

            # ---- phase 3: output projection ----
            with (
                tc.tile_pool(name="wo", bufs=1) as wop,
                tc.tile_pool(name="yst", bufs=4) as ystp,
                tc.tile_pool(name="pps3", bufs=4, space="PSUM") as pps3,
            ):
                for fo2 in range(2):
                    wts = []
                    for co in range(8):
                        wt_ = wop.tile([128, TQ], BF16, tag=f"wo{co}")
                        nc.sync.dma_start(
                            out=wt_[:],
                            in_=woT[128 * co : 128 * co + 128, TQ * fo2 : TQ * fo2 + TQ],
                        )
                        wts.append(wt_)
                    for to in range(8):
                        ps = pps3.tile([128, TQ], F32, tag="pps3")
                        for co in range(8):
                            nc.tensor.matmul(
                                ps[:],
                                oT[:, co, 128 * to : 128 * to + 128],
                                wts[co][:],
                                start=(co == 0),
                                stop=(co == 7),
                            )
                        yst = ystp.tile([128, TQ], F32, tag="yst")
                        nc.vector.tensor_tensor(
                            yst[:], ps[:], borep[:, TQ * fo2 : TQ * fo2 + TQ], ALU.add
                        )
                        nc.sync.dma_start(
                            out=y_out[128 * to : 128 * to + 128, TQ * fo2 : TQ * fo2 + TQ],
                            in_=yst[:],
                        )

    _split_multi_waits(nc)
    return nc


_NC_CACHE = None


def _get_nc():
    global _NC_CACHE
    if _NC_CACHE is None:
        _NC_CACHE = _build()
    return _NC_CACHE


def _bf(x):
    return np.ascontiguousarray(np.asarray(x, np.float32).astype(ml_dtypes.bfloat16))


def _prepare_in_maps(
    query, key_, value, Wq, bq, Wk, bk, Wv, bv, Wo, bo, bias_table, offset
):
    query = np.asarray(query, np.float32)
    key_ = np.asarray(key_, np.float32)
    value = np.asarray(value, np.float32)
    shared = {
        "wqT": _bf(np.asarray(Wq, np.float32).T),
        "wkT": _bf(np.asarray(Wk, np.float32).T),
        "wvT": _bf(np.asarray(Wv, np.float32).T),
        "woT": _bf(np.asarray(Wo, np.float32).T),
        "bq2": np.ascontiguousarray(np.asarray(bq, np.float32).reshape(8, 128).T),
        "bk2": np.ascontiguousarray(np.asarray(bk, np.float32).reshape(8, 128).T),
        "bv1": np.ascontiguousarray(np.asarray(bv, np.float32)),
        "bo1": np.ascontiguousarray(np.asarray(bo, np.float32)),
        "offs": np.ascontiguousarray(np.asarray(offset, np.float32)),
    }
    tab = np.asarray(bias_table, np.float32)  # [2T-1, H]
    pad = np.concatenate([tab[0:1], tab, tab[-1:]], axis=0)  # [2T+1, H]
    shared["rtabp"] = np.ascontiguousarray(pad[::-1].T)  # [H, 2T+1]

    in_maps = []
    for b in range(B):
        m = dict(shared)
        m["xqT"] = _bf(query[b].T)
        m["xkT"] = _bf(key_[b].T)
        m["xvT"] = _bf(value[b].T)
        in_maps.append(m)
    return in_maps


def kernel(**inputs):
    in_maps = _prepare_in_maps(
        inputs["query"], inputs["key_"], inputs["value"],
        inputs["Wq"], inputs["bq"], inputs["Wk"], inputs["bk"],
        inputs["Wv"], inputs["bv"], inputs["Wo"], inputs["bo"],
        inputs["bias_table"], inputs["offset"],
    )
    nc = _get_nc()
    res = run_bass_kernel_spmd(nc, in_maps, list(range(B)))
    out = np.stack([res.results[b]["y"] for b in range(B)], axis=0)
    return out.astype(np.float32)



# revision 9
# speedup vs baseline: 1.3878x; 1.3878x over previous
"""Trainium2 Bass kernel for CustomTemporalAttention.

B=8, T=1024, E=1024, H=16, D=64. Pure batch data-parallel across 8 cores.

v2: single interleaved schedule built to keep the PE array's HAM clock-gate
warm (the v1 trace showed the whole attention phase running at K=4/8 =
1.2 GHz with 240us of PE idle spread over >3.4us gaps):

  - heads processed in PAIRS (2p, 2p+1): their K=64 S matmuls land in row
    tiles (0,0)/(64,0) of the 64x128 PE tiling mode back-to-back, so the two
    heads' score matmuls stream concurrently (~2x S throughput).
  - q/k projection jobs are emitted between attention pairs; v jobs up
    front.  The PE queue always holds independent work, no gap > ~2us.
  - normalization: one DVE copy grabs num+den ([65,512] PSUM->SBUF bf16),
    den transposes through DRAM to [128,8] for a partition-parallel
    reciprocal, broadcast back as a [64,1024] stride-0 DMA, one bf16 DVE
    multiply per tq half.  No gpsimd compute, no PSUM reads off DVE/ACT
    critical path beyond the single copy.
  - PSUM budget: SP pool 2x[128,1024] (S tiles + proj/phase-3 accumulators)
    + OP pool 4x[65,512] (PV accumulators) = exactly 8 banks.

Math identical to v1: qT/kT channel-major [128,8,T]; S^T per tk chunk;
P^T = exp(0.125*S^T) * exp(bias) with the per-head exp(b) Toeplitz slab
(overlapping-window DMA, reversed free-dim reads); PV accumulates
[v_h | 1]^T @ P^T over tk chunks; y = O @ Wo^T + bo.
"""

import sys

sys.path.insert(0, "/opt/trn_rl_repo")

import ml_dtypes
import numpy as np

import concourse.bass as bass
import concourse.mybir as mybir
import concourse.tile as tile
from concourse.bass_utils import run_bass_kernel_spmd

F32 = mybir.dt.float32
BF16 = mybir.dt.bfloat16
AF = mybir.ActivationFunctionType
ALU = mybir.AluOpType

B, T, E, H = 8, 1024, 1024, 16
D = E // H  # 64
TQ = 512
W_BSP = 1920


def _split_multi_waits(nc):
    """This walrus build accepts at most one sync-wait per instruction; hoist
    extras onto same-engine NoOp carriers placed immediately before."""
    n = 0
    for f in nc.m.functions:
        for blk in f.blocks:
            out = []
            for inst in blk.instructions:
                si = inst.sync_info
                waits = list(si.on_wait) if si and si.on_wait else []
                if len(waits) > 1:
                    for w in waits[:-1]:
                        n += 1
                        nop = mybir.InstNoOp(name=f"{inst.name}-ws{n}", ins=[], outs=[])
                        nop.engine = inst.engine
                        nop.sync_info = mybir.SyncInfo(on_wait=[w], on_update=[])
                        out.append(nop)
                    inst.sync_info = mybir.SyncInfo(
                        on_wait=[waits[-1]], on_update=list(si.on_update or [])
                    )
                out.append(inst)
            blk.instructions = out
    return n


def _craft(ap, dims, offset=None):
    c = ap.copy()
    c.ap = ap.ap.__class__(dims)
    if offset is not None:
        c.offset = offset
    return c


def _build():
    nc = bass.Bass()

    xqT = nc.declare_dram_parameter("xqT", [E, T], BF16, isOutput=False)
    xkT = nc.declare_dram_parameter("xkT", [E, T], BF16, isOutput=False)
    xvT = nc.declare_dram_parameter("xvT", [E, T], BF16, isOutput=False)
    wqT = nc.declare_dram_parameter("wqT", [E, E], BF16, isOutput=False)
    wkT = nc.declare_dram_parameter("wkT", [E, E], BF16, isOutput=False)
    wvT = nc.declare_dram_parameter("wvT", [E, E], BF16, isOutput=False)
    woT = nc.declare_dram_parameter("woT", [E, E], BF16, isOutput=False)
    bq2 = nc.declare_dram_parameter("bq2", [128, 8], F32, isOutput=False)
    bk2 = nc.declare_dram_parameter("bk2", [128, 8], F32, isOutput=False)
    bv1 = nc.declare_dram_parameter("bv1", [E], F32, isOutput=False)
    bo1 = nc.declare_dram_parameter("bo1", [E], F32, isOutput=False)
    rtabp = nc.declare_dram_parameter("rtabp", [H, 2 * T + 1], F32, isOutput=False)
    offs = nc.declare_dram_parameter("offs", [1], F32, isOutput=False)
    y_out = nc.declare_dram_parameter("y", [T, E], F32, isOutput=True)

    with tile.TileContext(nc) as tc:
        with (
            tc.tile_pool(name="persist", bufs=1) as persist,
            tc.tile_pool(name="small", bufs=1) as small,
            tc.tile_pool(name="dram", bufs=1, space="DRAM") as drp,
        ):
            # persistent SBUF state
            xq = persist.tile([128, 8, T], BF16, tag="xq")
            xk = persist.tile([128, 8, T], BF16, tag="xk")
            xv = persist.tile([128, 8, T], BF16, tag="xv")
            qT = persist.tile([128, 8, T], BF16, tag="qT")
            kT = persist.tile([128, 8, T], BF16, tag="kT")
            vp = persist.tile([128, 8, H, D + 1], BF16, tag="vp")
            oT = persist.tile([128, 8, T], BF16, tag="oT")
            bvrep = persist.tile([128, E], F32, tag="bvrep")
            borep = persist.tile([128, E], F32, tag="borep")
            bqs = small.tile([128, 8], F32, tag="bqs")
            bks = small.tile([128, 8], F32, tag="bks")

            nc.sync.dma_start(out=bqs[:], in_=bq2[:])
            nc.sync.dma_start(out=bks[:], in_=bk2[:])
            nc.sync.dma_start(out=bvrep[:], in_=_craft(bv1[:], [[0, 128], [1, E]], 0))
            nc.sync.dma_start(out=borep[:], in_=_craft(bo1[:], [[0, 128], [1, E]], 0))

            # ---- phase 0: blended relative-position table (identical to v1) ----
            p0ctx = tc.tile_pool(name="p0", bufs=1)
            p0 = p0ctx.__enter__()
            tab = p0.tile([H, 2 * T + 1], F32, tag="tab")
            nc.sync.dma_start(out=tab[:], in_=rtabp[:])
            off_sb = p0.tile([1, 1], F32, tag="off")
            nc.sync.dma_start(out=off_sb[:], in_=offs[None, :])
            th = p0.tile([1, 1], F32, tag="th")
            nc.scalar.activation(th[:], off_sb[:], AF.Tanh)
            w8 = p0.tile([1, 1], F32, tag="w8")
            nc.vector.tensor_scalar_mul(w8[:], th[:], 4.0)  # 8*u = 4*tanh
            abc = p0.tile([1, 3], F32, tag="abc")
            nc.vector.tensor_scalar(abc[:, 0:1], w8[:], -1.0, 0.0, ALU.mult, ALU.max)
            nc.vector.tensor_scalar(abc[:, 2:3], w8[:], 1.0, 0.0, ALU.mult, ALU.max)
            tsum = p0.tile([1, 1], F32, tag="tsum")
            nc.vector.tensor_tensor(tsum[:], abc[:, 0:1], abc[:, 2:3], ALU.add)
            nc.vector.tensor_scalar(abc[:, 1:2], tsum[:], -1.0, 8.0, ALU.mult, ALU.add)
            abc_dram = drp.tile([3], F32, tag="abc_dram")
            nc.gpsimd.dma_start(out=abc_dram[None, :], in_=abc[:])
            abc16 = p0.tile([H, 3], F32, tag="abc16")
            nc.gpsimd.dma_start(out=abc16[:], in_=_craft(abc_dram[:], [[0, H], [1, 3]], 0))

            nblend = 2 * T - 1
            rb = p0.tile([H, nblend], F32, tag="rb")
            rb_t = p0.tile([H, nblend], F32, tag="rb_t")
            nc.vector.tensor_scalar(rb[:], tab[:, 2 : 2 + nblend], abc16[:, 0:1], None, ALU.mult)
            nc.vector.tensor_scalar(rb_t[:], tab[:, 1 : 1 + nblend], abc16[:, 1:2], None, ALU.mult)
            nc.vector.tensor_tensor(rb[:], rb[:], rb_t[:], ALU.add)
            nc.vector.tensor_scalar(rb_t[:], tab[:, 0:nblend], abc16[:, 2:3], None, ALU.mult)
            nc.vector.tensor_tensor(rb[:], rb[:], rb_t[:], ALU.add)
            erb = p0.tile([H, nblend], BF16, tag="erb")
            nc.scalar.activation(erb[:], rb[:], AF.Exp, scale=0.125)
            erb_dram = drp.tile([H, nblend], BF16, tag="erb_dram")
            nc.gpsimd.dma_start(out=erb_dram[:], in_=erb[:])
            p0ctx.__exit__(None, None, None)

            # bulk input loads (sync queue)
            for eo in range(8):
                nc.sync.dma_start(out=xq[:, eo, :], in_=xqT[128 * eo : 128 * eo + 128, :])
            for eo in range(8):
                nc.sync.dma_start(out=xk[:, eo, :], in_=xkT[128 * eo : 128 * eo + 128, :])
            for eo in range(8):
                nc.sync.dma_start(out=xv[:, eo, :], in_=xvT[128 * eo : 128 * eo + 128, :])

            with (
                tc.tile_pool(name="wt8", bufs=3) as wt8p,      # [128,8,128] w chunks (q/k)
                tc.tile_pool(name="wmv", bufs=1) as wmvp,      # [128,1024] moving w (v then o)
                tc.tile_pool(name="eb", bufs=3) as ebp,
                tc.tile_pool(name="pt", bufs=4) as ptp,
                tc.tile_pool(name="pt0", bufs=3) as pt0p,
                tc.tile_pool(name="onum", bufs=6) as onp,      # [65,512] bf16 num+den
                tc.tile_pool(name="onrm", bufs=2) as onrmp,    # odd-head bounce
                tc.tile_pool(name="rep", bufs=4) as repp,
                tc.tile_pool(name="sm8", bufs=8) as sm8p,      # [128,8] den8/rec8
                tc.tile_pool(name="yst", bufs=2) as ystp,
                tc.tile_pool(name="SP", bufs=2, space="PSUM") as spp,   # [128,1024] = 2 banks
                tc.tile_pool(name="OP", bufs=4, space="PSUM") as opp,   # [65,512]  = 1 bank
                tc.tile_pool(name="dr2", bufs=8, space="DRAM") as drp2,
            ):
                # ---------- projection job emitters ----------
                def emit_qk_job(name, x_sb, w_in, dst, bias_sb, fo):
                    wt8 = wt8p.tile([128, 8, 128], BF16, tag="wt8", name=f"w{name}{fo}")
                    nc.sync.dma_start(
                        out=wt8[:],
                        in_=w_in[:, 128 * fo : 128 * fo + 128].rearrange(
                            "(e p) f -> p e f", p=128
                        ),
                    )
                    sp = spp.tile([128, 2 * TQ], F32, tag="SP", name=f"p{name}{fo}")
                    for tqh in range(2):
                        for eo in range(8):
                            nc.tensor.matmul(
                                sp[:, TQ * tqh : TQ * tqh + TQ],
                                wt8[:, eo, :],
                                x_sb[:, eo, TQ * tqh : TQ * tqh + TQ],
                                start=(eo == 0),
                                stop=(eo == 7),
                            )
                    nc.vector.tensor_scalar(
                        dst[:, fo, :], sp[:], 1.0, bias_sb[:, fo : fo + 1],
                        ALU.mult, ALU.add,
                    )

                wv_tiles = {}

                def emit_v_job(to):
                    for eo in range(8):
                        if eo not in wv_tiles:
                            wt_ = wmvp.tile([128, 2 * TQ], BF16, tag=f"wmv{eo}", name=f"wv{eo}")
                            nc.sync.dma_start(
                                out=wt_[:], in_=wvT[128 * eo : 128 * eo + 128, :]
                            )
                            wv_tiles[eo] = wt_
                    sp = spp.tile([128, 2 * TQ], F32, tag="SP", name=f"pv{to}")
                    to2, toi = divmod(to, 4)
                    for fv in range(2):
                        for eo in range(8):
                            nc.tensor.matmul(
                                sp[:, TQ * fv : TQ * fv + TQ],
                                xv[:, eo, TQ * to2 + 128 * toi : TQ * to2 + 128 * toi + 128],
                                wv_tiles[eo][:, TQ * fv : TQ * fv + TQ],
                                start=(eo == 0),
                                stop=(eo == 7),
                            )
                    nc.vector.tensor_tensor(
                        vp[:, to, :, 0:D],
                        sp[:].rearrange("p (h d) -> p h d", d=D),
                        bvrep[:].rearrange("p (h d) -> p h d", d=D),
                        ALU.add,
                    )

                # ---------- attention pair machinery ----------
                pend = {}

                def emit_eb(hh):
                    eb_ = ebp.tile([128, W_BSP], BF16, tag="eb", name=f"eb{hh}")
                    nc.gpsimd.dma_start(
                        out=eb_[:],
                        in_=_craft(erb_dram[:], [[1, 128], [1, W_BSP]], hh * nblend),
                    )
                    return eb_

                def norm_stage_a(p):
                    st = pend[p]
                    for hi in range(2):
                        for tqh in range(2):
                            on_ = onp.tile([D + 1, TQ], BF16, tag="onum",
                                           name=f"on{p}_{hi}_{tqh}")
                            nc.vector.tensor_copy(out=on_[:], in_=st["ops"][hi][tqh][:])
                            st["onum"][hi].append(on_)
                        dd = drp2.tile([2 * TQ], BF16, tag="dend", name=f"dd{p}_{hi}")
                        for tqh in range(2):
                            nc.gpsimd.dma_start(
                                out=_craft(dd[None, :], [[0, 1], [1, TQ]], TQ * tqh),
                                in_=st["onum"][hi][tqh][D : D + 1, :],
                            )
                        st["dd"].append(dd)

                def norm_stage_b(p):
                    st = pend[p]
                    for hi in range(2):
                        d8 = sm8p.tile([128, 8], BF16, tag="d8", name=f"d8{p}_{hi}")
                        nc.gpsimd.dma_start(
                            out=d8[:], in_=st["dd"][hi].rearrange("(f p) -> p f", p=128)
                        )
                        r8 = sm8p.tile([128, 8], BF16, tag="r8", name=f"r8{p}_{hi}")
                        with nc.allow_low_precision(reason="bf16 softmax denom ~0.4% ok"):
                            nc.vector.reciprocal(r8[:], d8[:])
                        rd = drp2.tile([2 * TQ], BF16, tag="recd", name=f"rd{p}_{hi}")
                        nc.gpsimd.dma_start(
                            out=rd.rearrange("(f p) -> p f", p=128), in_=r8[:]
                        )
                        rp_ = repp.tile([D, 2 * TQ], BF16, tag="rep", name=f"rp{p}_{hi}")
                        nc.gpsimd.dma_start(
                            out=rp_[:], in_=_craft(rd[:], [[0, D], [1, 2 * TQ]], 0)
                        )
                        st["rep"].append(rp_)

                def norm_stage_c(p):
                    st = pend.pop(p)
                    for hi in range(2):
                        for tqh in range(2):
                            on_ = st["onum"][hi][tqh]
                            rp_ = st["rep"][hi]
                            if hi == 0:
                                nc.vector.tensor_tensor(
                                    oT[0:D, p, TQ * tqh : TQ * tqh + TQ],
                                    on_[0:D, :],
                                    rp_[:, TQ * tqh : TQ * tqh + TQ],
                                    ALU.mult,
                                )
                            else:
                                om = onrmp.tile([D, TQ], BF16, tag="onrm",
                                                name=f"om{p}_{tqh}")
                                nc.vector.tensor_tensor(
                                    om[:], on_[0:D, :],
                                    rp_[:, TQ * tqh : TQ * tqh + TQ], ALU.mult,
                                )
                                nc.gpsimd.dma_start(
                                    out=oT[D : 2 * D, p, TQ * tqh : TQ * tqh + TQ],
                                    in_=om[:],
                                )

                def emit_pair(p, hooks):
                    """hooks: dict chunk-index -> list of zero-arg emitters run
                    right before that chunk's S matmuls."""
                    hA, hB = 2 * p, 2 * p + 1
                    ebA, ebB = emit_eb(hA), emit_eb(hB)
                    ops = [
                        [opp.tile([D + 1, TQ], F32, tag="OP", name=f"op{p}_{hi}_{t}")
                         for t in range(2)]
                        for hi in range(2)
                    ]
                    pts = {}

                    def emit_pv(c):
                        ptA, ptB = pts.pop(c)
                        for hi, pt_ in ((0, ptA), (1, ptB)):
                            for tqh in range(2):
                                nc.tensor.matmul(
                                    ops[hi][tqh][:],
                                    vp[:, c, 2 * p + hi, :],
                                    pt_[:, TQ * tqh : TQ * tqh + TQ],
                                    start=(c == 0),
                                    stop=(c == 7),
                                )

                    for c in range(8):
                        for fn in hooks.get(c, ()):
                            fn()
                        sps = []
                        for hp0, hh in ((0, hA), (64, hB)):
                            sp = spp.tile([128, 2 * TQ], F32, tag="SP",
                                          name=f"s{hh}_{c}")
                            for tqh in range(2):
                                nc.tensor.matmul(
                                    sp[:, TQ * tqh : TQ * tqh + TQ],
                                    kT[hp0 : hp0 + 64, p, 128 * c : 128 * c + 128],
                                    qT[hp0 : hp0 + 64, p, TQ * tqh : TQ * tqh + TQ],
                                    start=True,
                                    stop=True,
                                )
                            sps.append(sp)
                        ptc = []
                        for hi, (sp, eb_) in enumerate(zip(sps, (ebA, ebB))):
                            pt0 = pt0p.tile([128, 2 * TQ], BF16, tag="pt0")
                            nc.scalar.activation(pt0[:], sp[:], AF.Exp, scale=0.125)
                            s0 = 1023 + 128 * c
                            pt_ = ptp.tile([128, 2 * TQ], BF16, tag="pt")
                            nc.vector.tensor_tensor(
                                pt_[:], pt0[:],
                                eb_[:, s0 - (2 * TQ - 1) : s0 + 1][:, ::-1],
                                ALU.mult,
                            )
                            ptc.append(pt_)
                        pts[c] = ptc
                        if c >= 1:
                            emit_pv(c - 1)
                    emit_pv(7)
                    pend[p] = {"ops": ops, "onum": [[], []], "dd": [], "rep": []}

                # ---------- schedule ----------
                emit_qk_job("q", xq, wqT, qT, bqs, 0)
                emit_qk_job("k", xk, wkT, kT, bks, 0)
                emit_qk_job("q", xq, wqT, qT, bqs, 1)
                emit_qk_job("k", xk, wkT, kT, bks, 1)
                for to in range(8):
                    emit_v_job(to)
                nc.vector.memset(vp[:, :, :, D : D + 1], 1.0)

                for p in range(8):
                    hooks = {}
                    if p >= 1:
                        hooks[1] = [lambda p=p: norm_stage_a(p - 1)]
                        hooks[3] = [lambda p=p: norm_stage_b(p - 1)]
                        hooks[5] = [lambda p=p: norm_stage_c(p - 1)]
                    if p + 2 < 8:
                        hooks.setdefault(6, []).append(
                            lambda p=p: emit_qk_job("q", xq, wqT, qT, bqs, p + 2)
                        )
                        hooks.setdefault(7, []).append(
                            lambda p=p: emit_qk_job("k", xk, wkT, kT, bks, p + 2)
                        )
                    emit_pair(p, hooks)
                norm_stage_a(7)
                norm_stage_b(7)
                norm_stage_c(7)

                # ---------- phase 3: output projection ----------
                wo_tiles = {}
                for co in range(8):
                    wt_ = wmvp.tile([128, 2 * TQ], BF16, tag=f"wmv{co}", name=f"wo{co}")
                    nc.sync.dma_start(out=wt_[:], in_=woT[128 * co : 128 * co + 128, :])
                    wo_tiles[co] = wt_
                for to in range(8):
                    sp = spp.tile([128, 2 * TQ], F32, tag="SP", name=f"y{to}")
                    for fh in range(2):
                        for co in range(8):
                            nc.tensor.matmul(
                                sp[:, TQ * fh : TQ * fh + TQ],
                                oT[:, co, 128 * to : 128 * to + 128],
                                wo_tiles[co][:, TQ * fh : TQ * fh + TQ],
                                start=(co == 0),
                                stop=(co == 7),
                            )
                    yst = ystp.tile([128, 2 * TQ], F32, tag="yst")
                    nc.vector.tensor_tensor(yst[:], sp[:], borep[:], ALU.add)
                    nc.sync.dma_start(
                        out=y_out[128 * to : 128 * to + 128, :], in_=yst[:]
                    )

    _split_multi_waits(nc)
    return nc


_NC_CACHE = None


def _get_nc():
    global _NC_CACHE
    if _NC_CACHE is None:
        _NC_CACHE = _build()
    return _NC_CACHE


def _bf(x):
    return np.ascontiguousarray(np.asarray(x, np.float32).astype(ml_dtypes.bfloat16))


def _prepare_in_maps(
    query, key_, value, Wq, bq, Wk, bk, Wv, bv, Wo, bo, bias_table, offset
):
    query = np.asarray(query, np.float32)
    key_ = np.asarray(key_, np.float32)
    value = np.asarray(value, np.float32)
    shared = {
        "wqT": _bf(np.asarray(Wq, np.float32).T),
        "wkT": _bf(np.asarray(Wk, np.float32).T),
        "wvT": _bf(np.asarray(Wv, np.float32).T),
        "woT": _bf(np.asarray(Wo, np.float32).T),
        "bq2": np.ascontiguousarray(np.asarray(bq, np.float32).reshape(8, 128).T),
        "bk2": np.ascontiguousarray(np.asarray(bk, np.float32).reshape(8, 128).T),
        "bv1": np.ascontiguousarray(np.asarray(bv, np.float32)),
        "bo1": np.ascontiguousarray(np.asarray(bo, np.float32)),
        "offs": np.ascontiguousarray(np.asarray(offset, np.float32)),
    }
    tab = np.asarray(bias_table, np.float32)  # [2T-1, H]
    pad = np.concatenate([tab[0:1], tab, tab[-1:]], axis=0)  # [2T+1, H]
    shared["rtabp"] = np.ascontiguousarray(pad[::-1].T)  # [H, 2T+1]

    in_maps = []
    for b in range(B):
        m = dict(shared)
        m["xqT"] = _bf(query[b].T)
        m["xkT"] = _bf(key_[b].T)
        m["xvT"] = _bf(value[b].T)
        in_maps.append(m)
    return in_maps


def kernel(**inputs):
    in_maps = _prepare_in_maps(
        inputs["query"], inputs["key_"], inputs["value"],
        inputs["Wq"], inputs["bq"], inputs["Wk"], inputs["bk"],
        inputs["Wv"], inputs["bv"], inputs["Wo"], inputs["bo"],
        inputs["bias_table"], inputs["offset"],
    )
    nc = _get_nc()
    res = run_bass_kernel_spmd(nc, in_maps, list(range(B)))
    out = np.stack([res.results[b]["y"] for b in range(B)], axis=0)
    return out.astype(np.float32)


# revision 14
# speedup vs baseline: 1.4417x; 1.0388x over previous
"""Trainium2 Bass kernel for CustomTemporalAttention.

B=8, T=1024, E=1024, H=16, D=64. Pure batch data-parallel across 8 cores.

v2: single interleaved schedule built to keep the PE array's HAM clock-gate
warm (the v1 trace showed the whole attention phase running at K=4/8 =
1.2 GHz with 240us of PE idle spread over >3.4us gaps):

  - heads processed in PAIRS (2p, 2p+1): their K=64 S matmuls land in row
    tiles (0,0)/(64,0) of the 64x128 PE tiling mode back-to-back, so the two
    heads' score matmuls stream concurrently (~2x S throughput).
  - q/k projection jobs are emitted between attention pairs; v jobs up
    front.  The PE queue always holds independent work, no gap > ~2us.
  - normalization: one DVE copy grabs num+den ([65,512] PSUM->SBUF bf16),
    den transposes through DRAM to [128,8] for a partition-parallel
    reciprocal, broadcast back as a [64,1024] stride-0 DMA, one bf16 DVE
    multiply per tq half.  No gpsimd compute, no PSUM reads off DVE/ACT
    critical path beyond the single copy.
  - PSUM budget: SP pool 2x[128,1024] (S tiles + proj/phase-3 accumulators)
    + OP pool 4x[65,512] (PV accumulators) = exactly 8 banks.

Math identical to v1: qT/kT channel-major [128,8,T]; S^T per tk chunk;
P^T = exp(0.125*S^T) * exp(bias) with the per-head exp(b) Toeplitz slab
(overlapping-window DMA, reversed free-dim reads); PV accumulates
[v_h | 1]^T @ P^T over tk chunks; y = O @ Wo^T + bo.
"""

import sys

sys.path.insert(0, "/opt/trn_rl_repo")

import ml_dtypes
import numpy as np

import concourse.bass as bass
import concourse.mybir as mybir
import concourse.tile as tile
from concourse.bass_utils import run_bass_kernel_spmd

F32 = mybir.dt.float32
BF16 = mybir.dt.bfloat16
AF = mybir.ActivationFunctionType
ALU = mybir.AluOpType

B, T, E, H = 8, 1024, 1024, 16
D = E // H  # 64
TQ = 512
W_BSP = 1920


def _split_multi_waits(nc):
    """This walrus build accepts at most one sync-wait per instruction; hoist
    extras onto same-engine NoOp carriers placed immediately before."""
    n = 0
    for f in nc.m.functions:
        for blk in f.blocks:
            out = []
            for inst in blk.instructions:
                si = inst.sync_info
                waits = list(si.on_wait) if si and si.on_wait else []
                if len(waits) > 1:
                    for w in waits[:-1]:
                        n += 1
                        nop = mybir.InstNoOp(name=f"{inst.name}-ws{n}", ins=[], outs=[])
                        nop.engine = inst.engine
                        nop.sync_info = mybir.SyncInfo(on_wait=[w], on_update=[])
                        out.append(nop)
                    inst.sync_info = mybir.SyncInfo(
                        on_wait=[waits[-1]], on_update=list(si.on_update or [])
                    )
                out.append(inst)
            blk.instructions = out
    return n


def _craft(ap, dims, offset=None):
    c = ap.copy()
    c.ap = ap.ap.__class__(dims)
    if offset is not None:
        c.offset = offset
    return c


def _build():
    nc = bass.Bass()

    xqT = nc.declare_dram_parameter("xqT", [E, T], BF16, isOutput=False)
    xkT = nc.declare_dram_parameter("xkT", [E, T], BF16, isOutput=False)
    xvT = nc.declare_dram_parameter("xvT", [E, T], BF16, isOutput=False)
    wqT = nc.declare_dram_parameter("wqT", [E, E], BF16, isOutput=False)
    wkT = nc.declare_dram_parameter("wkT", [E, E], BF16, isOutput=False)
    wvT = nc.declare_dram_parameter("wvT", [E, E], BF16, isOutput=False)
    woT = nc.declare_dram_parameter("woT", [E, E], BF16, isOutput=False)
    bq2 = nc.declare_dram_parameter("bq2", [128, 8], F32, isOutput=False)
    bk2 = nc.declare_dram_parameter("bk2", [128, 8], F32, isOutput=False)
    bv1 = nc.declare_dram_parameter("bv1", [E], F32, isOutput=False)
    bo1 = nc.declare_dram_parameter("bo1", [E], F32, isOutput=False)
    rtabp = nc.declare_dram_parameter("rtabp", [H, 2 * T + 1], F32, isOutput=False)
    offs = nc.declare_dram_parameter("offs", [1], F32, isOutput=False)
    y_out = nc.declare_dram_parameter("y", [T, E], F32, isOutput=True)

    with tile.TileContext(nc) as tc:
        with (
            tc.tile_pool(name="persist", bufs=1) as persist,
            tc.tile_pool(name="small", bufs=1) as small,
            tc.tile_pool(name="dram", bufs=1, space="DRAM") as drp,
        ):
            # persistent SBUF state
            xq = persist.tile([128, 8, T], BF16, tag="xq")
            xk = persist.tile([128, 8, T], BF16, tag="xk")
            xv = persist.tile([128, 8, T], BF16, tag="xv")
            qT = persist.tile([128, 8, T], BF16, tag="qT")
            kT = persist.tile([128, 8, T], BF16, tag="kT")
            vp = persist.tile([128, 8, H, D + 1], BF16, tag="vp")
            oT = persist.tile([128, 8, T], BF16, tag="oT")
            bvrep = persist.tile([128, E], F32, tag="bvrep")
            borep = persist.tile([128, E], F32, tag="borep")
            bqs = small.tile([128, 8], F32, tag="bqs")
            bks = small.tile([128, 8], F32, tag="bks")

            nc.sync.dma_start(out=bqs[:], in_=bq2[:])
            nc.sync.dma_start(out=bks[:], in_=bk2[:])
            nc.sync.dma_start(out=bvrep[:], in_=_craft(bv1[:], [[0, 128], [1, E]], 0))
            nc.sync.dma_start(out=borep[:], in_=_craft(bo1[:], [[0, 128], [1, E]], 0))

            # ---- phase 0: blended relative-position table (identical to v1) ----
            p0ctx = tc.tile_pool(name="p0", bufs=1)
            p0 = p0ctx.__enter__()
            tab = p0.tile([H, 2 * T + 1], F32, tag="tab")
            nc.sync.dma_start(out=tab[:], in_=rtabp[:])
            off_sb = p0.tile([1, 1], F32, tag="off")
            nc.sync.dma_start(out=off_sb[:], in_=offs[None, :])
            th = p0.tile([1, 1], F32, tag="th")
            nc.scalar.activation(th[:], off_sb[:], AF.Tanh)
            w8 = p0.tile([1, 1], F32, tag="w8")
            nc.vector.tensor_scalar_mul(w8[:], th[:], 4.0)  # 8*u = 4*tanh
            abc = p0.tile([1, 3], F32, tag="abc")
            nc.vector.tensor_scalar(abc[:, 0:1], w8[:], -1.0, 0.0, ALU.mult, ALU.max)
            nc.vector.tensor_scalar(abc[:, 2:3], w8[:], 1.0, 0.0, ALU.mult, ALU.max)
            tsum = p0.tile([1, 1], F32, tag="tsum")
            nc.vector.tensor_tensor(tsum[:], abc[:, 0:1], abc[:, 2:3], ALU.add)
            nc.vector.tensor_scalar(abc[:, 1:2], tsum[:], -1.0, 8.0, ALU.mult, ALU.add)
            abc_dram = drp.tile([3], F32, tag="abc_dram")
            nc.gpsimd.dma_start(out=abc_dram[None, :], in_=abc[:])
            abc16 = p0.tile([H, 3], F32, tag="abc16")
            nc.gpsimd.dma_start(out=abc16[:], in_=_craft(abc_dram[:], [[0, H], [1, 3]], 0))

            nblend = 2 * T - 1
            rb = p0.tile([H, nblend], F32, tag="rb")
            rb_t = p0.tile([H, nblend], F32, tag="rb_t")
            nc.vector.tensor_scalar(rb[:], tab[:, 2 : 2 + nblend], abc16[:, 0:1], None, ALU.mult)
            nc.vector.tensor_scalar(rb_t[:], tab[:, 1 : 1 + nblend], abc16[:, 1:2], None, ALU.mult)
            nc.vector.tensor_tensor(rb[:], rb[:], rb_t[:], ALU.add)
            nc.vector.tensor_scalar(rb_t[:], tab[:, 0:nblend], abc16[:, 2:3], None, ALU.mult)
            nc.vector.tensor_tensor(rb[:], rb[:], rb_t[:], ALU.add)
            erb = p0.tile([H, nblend], BF16, tag="erb")
            nc.scalar.activation(erb[:], rb[:], AF.Exp, scale=0.125)
            erb_dram = drp.tile([H, nblend], BF16, tag="erb_dram")
            nc.gpsimd.dma_start(out=erb_dram[:], in_=erb[:])
            p0ctx.__exit__(None, None, None)

            # bulk input loads (sync queue)
            for eo in range(8):
                nc.sync.dma_start(out=xq[:, eo, :], in_=xqT[128 * eo : 128 * eo + 128, :])
            for eo in range(8):
                nc.sync.dma_start(out=xk[:, eo, :], in_=xkT[128 * eo : 128 * eo + 128, :])
            for eo in range(8):
                nc.sync.dma_start(out=xv[:, eo, :], in_=xvT[128 * eo : 128 * eo + 128, :])

            with (
                tc.tile_pool(name="wt8", bufs=3) as wt8p,      # [128,8,128] w chunks (q/k)
                tc.tile_pool(name="wmv", bufs=1) as wmvp,      # [128,1024] moving w (v then o)
                tc.tile_pool(name="eb", bufs=4) as ebp,
                tc.tile_pool(name="pt", bufs=4) as ptp,
                tc.tile_pool(name="pt0", bufs=3) as pt0p,
                tc.tile_pool(name="onum", bufs=10) as onp,     # [65,512] bf16 num+den
                tc.tile_pool(name="onrm", bufs=2) as onrmp,    # odd-head bounce
                tc.tile_pool(name="rep", bufs=4) as repp,
                tc.tile_pool(name="sm8", bufs=8) as sm8p,      # [128,8] den8/rec8
                tc.tile_pool(name="yst", bufs=2) as ystp,
                tc.tile_pool(name="SP", bufs=2, space="PSUM") as spp,   # [128,1024] = 2 banks
                tc.tile_pool(name="OP", bufs=4, space="PSUM") as opp,   # [65,512]  = 1 bank
                tc.tile_pool(name="dr2", bufs=8, space="DRAM") as drp2,
            ):
                # ---------- projection job emitters ----------
                def emit_qk_job(name, x_sb, w_in, dst, bias_sb, fo):
                    wt8 = wt8p.tile([128, 8, 128], BF16, tag="wt8", name=f"w{name}{fo}")
                    nc.sync.dma_start(
                        out=wt8[:],
                        in_=w_in[:, 128 * fo : 128 * fo + 128].rearrange(
                            "(e p) f -> p e f", p=128
                        ),
                    )
                    sp = spp.tile([128, 2 * TQ], F32, tag="SP", name=f"p{name}{fo}")
                    for tqh in range(2):
                        for eo in range(8):
                            nc.tensor.matmul(
                                sp[:, TQ * tqh : TQ * tqh + TQ],
                                wt8[:, eo, :],
                                x_sb[:, eo, TQ * tqh : TQ * tqh + TQ],
                                start=(eo == 0),
                                stop=(eo == 7),
                            )
                    nc.vector.tensor_scalar(
                        dst[:, fo, :], sp[:], 1.0, bias_sb[:, fo : fo + 1],
                        ALU.mult, ALU.add,
                    )

                wv_tiles = {}

                def emit_v_job(to):
                    for eo in range(8):
                        if eo not in wv_tiles:
                            wt_ = wmvp.tile([128, 2 * TQ], BF16, tag=f"wmv{eo}", name=f"wv{eo}")
                            nc.sync.dma_start(
                                out=wt_[:], in_=wvT[128 * eo : 128 * eo + 128, :]
                            )
                            wv_tiles[eo] = wt_
                    sp = spp.tile([128, 2 * TQ], F32, tag="SP", name=f"pv{to}")
                    to2, toi = divmod(to, 4)
                    for fv in range(2):
                        for eo in range(8):
                            nc.tensor.matmul(
                                sp[:, TQ * fv : TQ * fv + TQ],
                                xv[:, eo, TQ * to2 + 128 * toi : TQ * to2 + 128 * toi + 128],
                                wv_tiles[eo][:, TQ * fv : TQ * fv + TQ],
                                start=(eo == 0),
                                stop=(eo == 7),
                            )
                    nc.vector.tensor_tensor(
                        vp[:, to, :, 0:D],
                        sp[:].rearrange("p (h d) -> p h d", d=D),
                        bvrep[:].rearrange("p (h d) -> p h d", d=D),
                        ALU.add,
                    )

                # ---------- attention pair machinery ----------
                pend = {}

                ebs = {}

                def emit_eb(hh):
                    eb_ = ebp.tile([128, W_BSP], BF16, tag="eb", name=f"eb{hh}")
                    nc.sync.dma_start(
                        out=eb_[:],
                        in_=_craft(erb_dram[:], [[1, 128], [1, W_BSP]], hh * nblend),
                    )
                    ebs[hh] = eb_

                def norm_stage_a(p):
                    st = pend[p]
                    for hi in range(2):
                        for tqh in range(2):
                            on_ = onp.tile([D + 1, TQ], BF16, tag="onum",
                                           name=f"on{p}_{hi}_{tqh}")
                            nc.vector.tensor_copy(out=on_[:], in_=st["ops"][hi][tqh][:])
                            st["onum"][hi].append(on_)
                        dd = drp2.tile([2 * TQ], BF16, tag="dend", name=f"dd{p}_{hi}")
                        for tqh in range(2):
                            nc.gpsimd.dma_start(
                                out=_craft(dd[None, :], [[0, 1], [1, TQ]], TQ * tqh),
                                in_=st["onum"][hi][tqh][D : D + 1, :],
                            )
                        st["dd"].append(dd)

                def norm_stage_b(p):
                    st = pend[p]
                    for hi in range(2):
                        d8 = sm8p.tile([128, 8], BF16, tag="d8", name=f"d8{p}_{hi}")
                        nc.gpsimd.dma_start(
                            out=d8[:], in_=st["dd"][hi].rearrange("(f p) -> p f", p=128)
                        )
                        r8 = sm8p.tile([128, 8], BF16, tag="r8", name=f"r8{p}_{hi}")
                        with nc.allow_low_precision(reason="bf16 softmax denom ~0.4% ok"):
                            nc.vector.reciprocal(r8[:], d8[:])
                        rd = drp2.tile([2 * TQ], BF16, tag="recd", name=f"rd{p}_{hi}")
                        nc.gpsimd.dma_start(
                            out=rd.rearrange("(f p) -> p f", p=128), in_=r8[:]
                        )
                        rp_ = repp.tile([D, 2 * TQ], BF16, tag="rep", name=f"rp{p}_{hi}")
                        nc.gpsimd.dma_start(
                            out=rp_[:], in_=_craft(rd[:], [[0, D], [1, 2 * TQ]], 0)
                        )
                        st["rep"].append(rp_)

                def norm_stage_c(p):
                    st = pend.pop(p)
                    for hi in range(2):
                        for tqh in range(2):
                            on_ = st["onum"][hi][tqh]
                            rp_ = st["rep"][hi]
                            if hi == 0:
                                nc.vector.tensor_tensor(
                                    oT[0:D, p, TQ * tqh : TQ * tqh + TQ],
                                    on_[0:D, :],
                                    rp_[:, TQ * tqh : TQ * tqh + TQ],
                                    ALU.mult,
                                )
                            else:
                                om = onrmp.tile([D, TQ], BF16, tag="onrm",
                                                name=f"om{p}_{tqh}")
                                nc.vector.tensor_tensor(
                                    om[:], on_[0:D, :],
                                    rp_[:, TQ * tqh : TQ * tqh + TQ], ALU.mult,
                                )
                                nc.gpsimd.dma_start(
                                    out=oT[D : 2 * D, p, TQ * tqh : TQ * tqh + TQ],
                                    in_=om[:],
                                )

                def emit_pair(p, hooks):
                    """hooks: dict chunk-index -> list of zero-arg emitters run
                    right before that chunk's S matmuls."""
                    hA, hB = 2 * p, 2 * p + 1
                    ebA, ebB = ebs.pop(hA), ebs.pop(hB)
                    ops = [
                        [opp.tile([D + 1, TQ], F32, tag="OP", name=f"op{p}_{hi}_{t}")
                         for t in range(2)]
                        for hi in range(2)
                    ]
                    pts = {}

                    def emit_pv(c):
                        ptA, ptB = pts.pop(c)
                        for hi, pt_ in ((0, ptA), (1, ptB)):
                            for tqh in range(2):
                                nc.tensor.matmul(
                                    ops[hi][tqh][:],
                                    vp[:, c, 2 * p + hi, :],
                                    pt_[:, TQ * tqh : TQ * tqh + TQ],
                                    start=(c == 0),
                                    stop=(c == 7),
                                )

                    for c in range(8):
                        for fn in hooks.get(c, ()):
                            fn()
                        sps = []
                        for hp0, hh in ((0, hA), (64, hB)):
                            sp = spp.tile([128, 2 * TQ], F32, tag="SP",
                                          name=f"s{hh}_{c}")
                            for tqh in range(2):
                                nc.tensor.matmul(
                                    sp[:, TQ * tqh : TQ * tqh + TQ],
                                    kT[hp0 : hp0 + 64, p, 128 * c : 128 * c + 128],
                                    qT[hp0 : hp0 + 64, p, TQ * tqh : TQ * tqh + TQ],
                                    start=True,
                                    stop=True,
                                )
                            sps.append(sp)
                        ptc = []
                        for hi, (sp, eb_) in enumerate(zip(sps, (ebA, ebB))):
                            pt0 = pt0p.tile([128, 2 * TQ], BF16, tag="pt0")
                            nc.scalar.activation(pt0[:], sp[:], AF.Exp, scale=0.125)
                            s0 = 1023 + 128 * c
                            pt_ = ptp.tile([128, 2 * TQ], BF16, tag="pt")
                            nc.vector.tensor_tensor(
                                pt_[:], pt0[:],
                                eb_[:, s0 - (2 * TQ - 1) : s0 + 1][:, ::-1],
                                ALU.mult,
                            )
                            ptc.append(pt_)
                        pts[c] = ptc
                        if c >= 1:
                            emit_pv(c - 1)
                    emit_pv(7)
                    pend[p] = {"ops": ops, "onum": [[], []], "dd": [], "rep": []}

                # ---------- schedule ----------
                emit_qk_job("q", xq, wqT, qT, bqs, 0)
                emit_qk_job("k", xk, wkT, kT, bks, 0)
                emit_v_job(0)
                emit_v_job(1)
                nc.vector.memset(vp[:, :, :, D : D + 1], 1.0)
                for hh in range(4):  # eb slabs for pairs 0 and 1
                    emit_eb(hh)

                def add_hook(hooks, c, fn):
                    hooks.setdefault(c, []).append(fn)

                for p in range(8):
                    hooks = {}
                    if p + 1 < 8:
                        add_hook(hooks, 0, lambda p=p: emit_eb(2 * p + 2))
                        add_hook(hooks, 0, lambda p=p: emit_eb(2 * p + 3))
                    if p == 0:
                        for j, c in ((2, 0), (3, 1), (4, 2), (5, 3), (6, 4), (7, 5)):
                            add_hook(hooks, c, lambda j=j: emit_v_job(j))
                    if p >= 1:
                        add_hook(hooks, 1, lambda p=p: norm_stage_a(p - 1))
                        add_hook(hooks, 5, lambda p=p: norm_stage_b(p - 1))
                    if p >= 2:
                        add_hook(hooks, 3, lambda p=p: norm_stage_c(p - 2))
                    if p + 1 < 8:
                        add_hook(hooks, 6,
                                 lambda p=p: emit_qk_job("q", xq, wqT, qT, bqs, p + 1))
                        add_hook(hooks, 7,
                                 lambda p=p: emit_qk_job("k", xk, wkT, kT, bks, p + 1))
                    emit_pair(p, hooks)
                norm_stage_c(6)
                norm_stage_a(7)
                norm_stage_b(7)
                norm_stage_c(7)

                # ---------- phase 3: output projection ----------
                wo_tiles = {}
                for co in range(8):
                    wt_ = wmvp.tile([128, 2 * TQ], BF16, tag=f"wmv{co}", name=f"wo{co}")
                    nc.sync.dma_start(out=wt_[:], in_=woT[128 * co : 128 * co + 128, :])
                    wo_tiles[co] = wt_
                for to in range(8):
                    sp = spp.tile([128, 2 * TQ], F32, tag="SP", name=f"y{to}")
                    for fh in range(2):
                        for co in range(8):
                            nc.tensor.matmul(
                                sp[:, TQ * fh : TQ * fh + TQ],
                                oT[:, co, 128 * to : 128 * to + 128],
                                wo_tiles[co][:, TQ * fh : TQ * fh + TQ],
                                start=(co == 0),
                                stop=(co == 7),
                            )
                    yst = ystp.tile([128, 2 * TQ], F32, tag="yst")
                    nc.vector.tensor_tensor(yst[:], sp[:], borep[:], ALU.add)
                    nc.sync.dma_start(
                        out=y_out[128 * to : 128 * to + 128, :], in_=yst[:]
                    )

    _split_multi_waits(nc)
    return nc


_NC_CACHE = None


def _get_nc():
    global _NC_CACHE
    if _NC_CACHE is None:
        _NC_CACHE = _build()
    return _NC_CACHE


def _bf(x):
    return np.ascontiguousarray(np.asarray(x, np.float32).astype(ml_dtypes.bfloat16))


def _prepare_in_maps(
    query, key_, value, Wq, bq, Wk, bk, Wv, bv, Wo, bo, bias_table, offset
):
    query = np.asarray(query, np.float32)
    key_ = np.asarray(key_, np.float32)
    value = np.asarray(value, np.float32)
    shared = {
        "wqT": _bf(np.asarray(Wq, np.float32).T),
        "wkT": _bf(np.asarray(Wk, np.float32).T),
        "wvT": _bf(np.asarray(Wv, np.float32).T),
        "woT": _bf(np.asarray(Wo, np.float32).T),
        "bq2": np.ascontiguousarray(np.asarray(bq, np.float32).reshape(8, 128).T),
        "bk2": np.ascontiguousarray(np.asarray(bk, np.float32).reshape(8, 128).T),
        "bv1": np.ascontiguousarray(np.asarray(bv, np.float32)),
        "bo1": np.ascontiguousarray(np.asarray(bo, np.float32)),
        "offs": np.ascontiguousarray(np.asarray(offset, np.float32)),
    }
    tab = np.asarray(bias_table, np.float32)  # [2T-1, H]
    pad = np.concatenate([tab[0:1], tab, tab[-1:]], axis=0)  # [2T+1, H]
    shared["rtabp"] = np.ascontiguousarray(pad[::-1].T)  # [H, 2T+1]

    in_maps = []
    for b in range(B):
        m = dict(shared)
        m["xqT"] = _bf(query[b].T)
        m["xkT"] = _bf(key_[b].T)
        m["xvT"] = _bf(value[b].T)
        in_maps.append(m)
    return in_maps


def kernel(**inputs):
    in_maps = _prepare_in_maps(
        inputs["query"], inputs["key_"], inputs["value"],
        inputs["Wq"], inputs["bq"], inputs["Wk"], inputs["bk"],
        inputs["Wv"], inputs["bv"], inputs["Wo"], inputs["bo"],
        inputs["bias_table"], inputs["offset"],
    )
    nc = _get_nc()
    res = run_bass_kernel_spmd(nc, in_maps, list(range(B)))
    out = np.stack([res.results[b]["y"] for b in range(B)], axis=0)
    return out.astype(np.float32)


# revision 20
# speedup vs baseline: 1.4976x; 1.0388x over previous
"""Trainium2 Bass kernel for CustomTemporalAttention.

B=8, T=1024, E=1024, H=16, D=64. Pure batch data-parallel across 8 cores.

v2: single interleaved schedule built to keep the PE array's HAM clock-gate
warm (the v1 trace showed the whole attention phase running at K=4/8 =
1.2 GHz with 240us of PE idle spread over >3.4us gaps):

  - heads processed in PAIRS (2p, 2p+1): their K=64 S matmuls land in row
    tiles (0,0)/(64,0) of the 64x128 PE tiling mode back-to-back, so the two
    heads' score matmuls stream concurrently (~2x S throughput).
  - q/k projection jobs are emitted between attention pairs; v jobs up
    front.  The PE queue always holds independent work, no gap > ~2us.
  - normalization: one DVE copy grabs num+den ([65,512] PSUM->SBUF bf16),
    den transposes through DRAM to [128,8] for a partition-parallel
    reciprocal, broadcast back as a [64,1024] stride-0 DMA, one bf16 DVE
    multiply per tq half.  No gpsimd compute, no PSUM reads off DVE/ACT
    critical path beyond the single copy.
  - PSUM budget: SP pool 2x[128,1024] (S tiles + proj/phase-3 accumulators)
    + OP pool 4x[65,512] (PV accumulators) = exactly 8 banks.

Math identical to v1: qT/kT channel-major [128,8,T]; S^T per tk chunk;
P^T = exp(0.125*S^T) * exp(bias) with the per-head exp(b) Toeplitz slab
(overlapping-window DMA, reversed free-dim reads); PV accumulates
[v_h | 1]^T @ P^T over tk chunks; y = O @ Wo^T + bo.
"""

import sys

sys.path.insert(0, "/opt/trn_rl_repo")

import ml_dtypes
import numpy as np

import concourse.bass as bass
import concourse.mybir as mybir
import concourse.tile as tile
from concourse.bass_utils import run_bass_kernel_spmd

F32 = mybir.dt.float32
BF16 = mybir.dt.bfloat16
AF = mybir.ActivationFunctionType
ALU = mybir.AluOpType

B, T, E, H = 8, 1024, 1024, 16
D = E // H  # 64
TQ = 512
W_BSP = 1920


def _split_multi_waits(nc):
    """This walrus build accepts at most one sync-wait per instruction; hoist
    extras onto same-engine NoOp carriers placed immediately before."""
    n = 0
    for f in nc.m.functions:
        for blk in f.blocks:
            out = []
            for inst in blk.instructions:
                si = inst.sync_info
                waits = list(si.on_wait) if si and si.on_wait else []
                if len(waits) > 1:
                    for w in waits[:-1]:
                        n += 1
                        nop = mybir.InstNoOp(name=f"{inst.name}-ws{n}", ins=[], outs=[])
                        nop.engine = inst.engine
                        nop.sync_info = mybir.SyncInfo(on_wait=[w], on_update=[])
                        out.append(nop)
                    inst.sync_info = mybir.SyncInfo(
                        on_wait=[waits[-1]], on_update=list(si.on_update or [])
                    )
                out.append(inst)
            blk.instructions = out
    return n


def _craft(ap, dims, offset=None):
    c = ap.copy()
    c.ap = ap.ap.__class__(dims)
    if offset is not None:
        c.offset = offset
    return c


def _build():
    nc = bass.Bass()

    xqT = nc.declare_dram_parameter("xqT", [E, T], BF16, isOutput=False)
    xkT = nc.declare_dram_parameter("xkT", [E, T], BF16, isOutput=False)
    xvT = nc.declare_dram_parameter("xvT", [E, T], BF16, isOutput=False)
    wqT = nc.declare_dram_parameter("wqT", [E, E], BF16, isOutput=False)
    wkT = nc.declare_dram_parameter("wkT", [E, E], BF16, isOutput=False)
    wvT = nc.declare_dram_parameter("wvT", [E, E], BF16, isOutput=False)
    woT = nc.declare_dram_parameter("woT", [E, E], BF16, isOutput=False)
    bq2 = nc.declare_dram_parameter("bq2", [128, 8], F32, isOutput=False)
    bk2 = nc.declare_dram_parameter("bk2", [128, 8], F32, isOutput=False)
    bv1 = nc.declare_dram_parameter("bv1", [E], F32, isOutput=False)
    bo1 = nc.declare_dram_parameter("bo1", [E], F32, isOutput=False)
    rtabp = nc.declare_dram_parameter("rtabp", [H, 2 * T + 1], F32, isOutput=False)
    offs = nc.declare_dram_parameter("offs", [1], F32, isOutput=False)
    y_out = nc.declare_dram_parameter("y", [T, E], F32, isOutput=True)

    with tile.TileContext(nc) as tc:
        with (
            tc.tile_pool(name="persist", bufs=1) as persist,
            tc.tile_pool(name="small", bufs=1) as small,
            tc.tile_pool(name="dram", bufs=1, space="DRAM") as drp,
        ):
            # persistent SBUF state
            xq = persist.tile([128, 8, T], BF16, tag="xq")
            xk = persist.tile([128, 8, T], BF16, tag="xk")
            xv = persist.tile([128, 8, T], BF16, tag="xv")
            qT = persist.tile([128, 8, T], BF16, tag="qT")
            kT = persist.tile([128, 8, T], BF16, tag="kT")
            vp = persist.tile([128, 8, H, D + 1], BF16, tag="vp")
            oT = persist.tile([128, 8, T], BF16, tag="oT")
            bvrep = persist.tile([128, E], F32, tag="bvrep")
            borep = persist.tile([128, E], F32, tag="borep")
            bqs = small.tile([128, 8], F32, tag="bqs")
            bks = small.tile([128, 8], F32, tag="bks")
            ones8 = small.tile([128, 8], BF16, tag="ones8")
            nc.vector.memset(ones8[:], 1.0)

            nc.sync.dma_start(out=bqs[:], in_=bq2[:])
            nc.sync.dma_start(out=bks[:], in_=bk2[:])
            nc.sync.dma_start(out=bvrep[:], in_=_craft(bv1[:], [[0, 128], [1, E]], 0))
            nc.sync.dma_start(out=borep[:], in_=_craft(bo1[:], [[0, 128], [1, E]], 0))

            # ---- phase 0: blended relative-position table (identical to v1) ----
            p0ctx = tc.tile_pool(name="p0", bufs=1)
            p0 = p0ctx.__enter__()
            tab = p0.tile([H, 2 * T + 1], F32, tag="tab")
            nc.sync.dma_start(out=tab[:], in_=rtabp[:])
            off_sb = p0.tile([1, 1], F32, tag="off")
            nc.sync.dma_start(out=off_sb[:], in_=offs[None, :])
            th = p0.tile([1, 1], F32, tag="th")
            nc.scalar.activation(th[:], off_sb[:], AF.Tanh)
            w8 = p0.tile([1, 1], F32, tag="w8")
            nc.vector.tensor_scalar_mul(w8[:], th[:], 4.0)  # 8*u = 4*tanh
            abc = p0.tile([1, 3], F32, tag="abc")
            nc.vector.tensor_scalar(abc[:, 0:1], w8[:], -1.0, 0.0, ALU.mult, ALU.max)
            nc.vector.tensor_scalar(abc[:, 2:3], w8[:], 1.0, 0.0, ALU.mult, ALU.max)
            tsum = p0.tile([1, 1], F32, tag="tsum")
            nc.vector.tensor_tensor(tsum[:], abc[:, 0:1], abc[:, 2:3], ALU.add)
            nc.vector.tensor_scalar(abc[:, 1:2], tsum[:], -1.0, 8.0, ALU.mult, ALU.add)
            abc_dram = drp.tile([3], F32, tag="abc_dram")
            nc.gpsimd.dma_start(out=abc_dram[None, :], in_=abc[:])
            abc16 = p0.tile([H, 3], F32, tag="abc16")
            nc.gpsimd.dma_start(out=abc16[:], in_=_craft(abc_dram[:], [[0, H], [1, 3]], 0))

            nblend = 2 * T - 1
            rb = p0.tile([H, nblend], F32, tag="rb")
            rb_t = p0.tile([H, nblend], F32, tag="rb_t")
            nc.vector.tensor_scalar(rb[:], tab[:, 2 : 2 + nblend], abc16[:, 0:1], None, ALU.mult)
            nc.vector.tensor_scalar(rb_t[:], tab[:, 1 : 1 + nblend], abc16[:, 1:2], None, ALU.mult)
            nc.vector.tensor_tensor(rb[:], rb[:], rb_t[:], ALU.add)
            nc.vector.tensor_scalar(rb_t[:], tab[:, 0:nblend], abc16[:, 2:3], None, ALU.mult)
            nc.vector.tensor_tensor(rb[:], rb[:], rb_t[:], ALU.add)
            erb = p0.tile([H, nblend], BF16, tag="erb")
            nc.scalar.activation(erb[:], rb[:], AF.Exp, scale=0.125)
            erb_dram = drp.tile([H, nblend], BF16, tag="erb_dram")
            nc.gpsimd.dma_start(out=erb_dram[:], in_=erb[:])
            p0ctx.__exit__(None, None, None)

            # bulk input loads (sync queue)
            for eo in range(8):
                nc.sync.dma_start(out=xq[:, eo, :], in_=xqT[128 * eo : 128 * eo + 128, :])
            for eo in range(8):
                nc.sync.dma_start(out=xk[:, eo, :], in_=xkT[128 * eo : 128 * eo + 128, :])
            for eo in range(8):
                nc.sync.dma_start(out=xv[:, eo, :], in_=xvT[128 * eo : 128 * eo + 128, :])

            with (
                tc.tile_pool(name="wt8", bufs=3) as wt8p,      # [128,8,128] w chunks (q/k)
                tc.tile_pool(name="wmv", bufs=1) as wmvp,      # [128,1024] moving w (v then o)
                tc.tile_pool(name="eb", bufs=4) as ebp,
                tc.tile_pool(name="pt", bufs=4) as ptp,
                tc.tile_pool(name="pt0", bufs=3) as pt0p,
                tc.tile_pool(name="onum", bufs=10) as onp,     # [65,512] bf16 num+den
                tc.tile_pool(name="onrm", bufs=2) as onrmp,    # odd-head bounce
                tc.tile_pool(name="rep", bufs=4) as repp,
                tc.tile_pool(name="sm8", bufs=8) as sm8p,      # [128,8] den8/rec8
                tc.tile_pool(name="yst", bufs=2) as ystp,
                tc.tile_pool(name="SP", bufs=2, space="PSUM") as spp,   # [128,1024] = 2 banks
                tc.tile_pool(name="OP", bufs=4, space="PSUM") as opp,   # [65,512]  = 1 bank
                tc.tile_pool(name="dr2", bufs=8, space="DRAM") as drp2,
            ):
                # ---------- projection job emitters ----------
                def emit_qk_job(name, x_sb, w_in, dst, bias_sb, fo):
                    wt8 = wt8p.tile([128, 8, 128], BF16, tag="wt8", name=f"w{name}{fo}")
                    nc.gpsimd.dma_start(
                        out=wt8[:],
                        in_=w_in[:, 128 * fo : 128 * fo + 128].rearrange(
                            "(e p) f -> p e f", p=128
                        ),
                    )
                    sp = spp.tile([128, 2 * TQ], F32, tag="SP", name=f"p{name}{fo}")
                    for tqh in range(2):
                        for eo in range(8):
                            nc.tensor.matmul(
                                sp[:, TQ * tqh : TQ * tqh + TQ],
                                wt8[:, eo, :],
                                x_sb[:, eo, TQ * tqh : TQ * tqh + TQ],
                                start=(eo == 0),
                                stop=(eo == 7),
                            )
                    nc.vector.tensor_scalar(
                        dst[:, fo, :], sp[:], 1.0, bias_sb[:, fo : fo + 1],
                        ALU.mult, ALU.add,
                    )

                wv_tiles = {}

                def emit_v_job(to):
                    for eo in range(8):
                        if eo not in wv_tiles:
                            wt_ = wmvp.tile([128, 2 * TQ], BF16, tag=f"wmv{eo}", name=f"wv{eo}")
                            nc.sync.dma_start(
                                out=wt_[:], in_=wvT[128 * eo : 128 * eo + 128, :]
                            )
                            wv_tiles[eo] = wt_
                    sp = spp.tile([128, 2 * TQ], F32, tag="SP", name=f"pv{to}")
                    to2, toi = divmod(to, 4)
                    for fv in range(2):
                        for eo in range(8):
                            nc.tensor.matmul(
                                sp[:, TQ * fv : TQ * fv + TQ],
                                xv[:, eo, TQ * to2 + 128 * toi : TQ * to2 + 128 * toi + 128],
                                wv_tiles[eo][:, TQ * fv : TQ * fv + TQ],
                                start=(eo == 0),
                                stop=(eo == 7),
                            )
                    nc.vector.tensor_tensor(
                        vp[:, to, :, 0:D],
                        sp[:].rearrange("p (h d) -> p h d", d=D),
                        bvrep[:].rearrange("p (h d) -> p h d", d=D),
                        ALU.add,
                    )

                # ---------- attention pair machinery ----------
                pend = {}

                ebs = {}

                def emit_eb(hh):
                    eb_ = ebp.tile([128, W_BSP], BF16, tag="eb", name=f"eb{hh}")
                    nc.sync.dma_start(
                        out=eb_[:],
                        in_=_craft(erb_dram[:], [[1, 128], [1, W_BSP]], hh * nblend),
                    )
                    ebs[hh] = eb_

                def norm_stage_a(p):
                    st = pend[p]
                    for hi in range(2):
                        for tqh in range(2):
                            on_ = onp.tile([D + 1, TQ], BF16, tag="onum",
                                           name=f"on{p}_{hi}_{tqh}")
                            nc.vector.tensor_copy(out=on_[:], in_=st["ops"][hi][tqh][:])
                            st["onum"][hi].append(on_)
                        dd = drp2.tile([2 * TQ], BF16, tag="dend", name=f"dd{p}_{hi}")
                        for tqh in range(2):
                            nc.gpsimd.dma_start(
                                out=_craft(dd[None, :], [[0, 1], [1, TQ]], TQ * tqh),
                                in_=st["onum"][hi][tqh][D : D + 1, :],
                            )
                        st["dd"].append(dd)

                def norm_stage_b(p):
                    st = pend[p]
                    for hi in range(2):
                        d8 = sm8p.tile([128, 8], BF16, tag="d8", name=f"d8{p}_{hi}")
                        nc.gpsimd.dma_start(
                            out=d8[:], in_=st["dd"][hi].rearrange("(f p) -> p f", p=128)
                        )
                        r8 = sm8p.tile([128, 8], BF16, tag="r8", name=f"r8{p}_{hi}")
                        with nc.allow_low_precision(reason="bf16 softmax denom ~0.4% ok"):
                            nc.vector.reciprocal(r8[:], d8[:])
                        rd = drp2.tile([2 * TQ], BF16, tag="recd", name=f"rd{p}_{hi}")
                        nc.gpsimd.dma_start(
                            out=rd.rearrange("(f p) -> p f", p=128), in_=r8[:]
                        )
                        rp_ = repp.tile([D, 2 * TQ], BF16, tag="rep", name=f"rp{p}_{hi}")
                        nc.gpsimd.dma_start(
                            out=rp_[:], in_=_craft(rd[:], [[0, D], [1, 2 * TQ]], 0)
                        )
                        st["rep"].append(rp_)

                def norm_stage_c(p):
                    st = pend.pop(p)
                    for hi in range(2):
                        for tqh in range(2):
                            on_ = st["onum"][hi][tqh]
                            rp_ = st["rep"][hi]
                            if hi == 0:
                                nc.gpsimd.tensor_tensor(
                                    oT[0:D, p, TQ * tqh : TQ * tqh + TQ],
                                    on_[0:D, :],
                                    rp_[:, TQ * tqh : TQ * tqh + TQ],
                                    ALU.mult,
                                )
                            else:
                                om = onrmp.tile([D, TQ], BF16, tag="onrm",
                                                name=f"om{p}_{tqh}")
                                nc.gpsimd.tensor_tensor(
                                    om[:], on_[0:D, :],
                                    rp_[:, TQ * tqh : TQ * tqh + TQ], ALU.mult,
                                )
                                nc.gpsimd.dma_start(
                                    out=oT[D : 2 * D, p, TQ * tqh : TQ * tqh + TQ],
                                    in_=om[:],
                                )

                def emit_pair(p, hooks):
                    """hooks: dict chunk-index -> list of zero-arg emitters run
                    right before that chunk's S matmuls."""
                    hA, hB = 2 * p, 2 * p + 1
                    ebA, ebB = ebs.pop(hA), ebs.pop(hB)
                    ops = [
                        [opp.tile([D + 1, TQ], F32, tag="OP", name=f"op{p}_{hi}_{t}")
                         for t in range(2)]
                        for hi in range(2)
                    ]
                    pts = {}

                    def emit_pv(c):
                        ptA, ptB = pts.pop(c)
                        for hi, pt_ in ((0, ptA), (1, ptB)):
                            for tqh in range(2):
                                nc.tensor.matmul(
                                    ops[hi][tqh][:],
                                    vp[:, c, 2 * p + hi, :],
                                    pt_[:, TQ * tqh : TQ * tqh + TQ],
                                    start=(c == 0),
                                    stop=(c == 7),
                                )

                    for c in range(8):
                        for fn in hooks.get(c, ()):
                            fn()
                        sps = []
                        for hp0, hh in ((0, hA), (64, hB)):
                            sp = spp.tile([128, 2 * TQ], F32, tag="SP",
                                          name=f"s{hh}_{c}")
                            for tqh in range(2):
                                nc.tensor.matmul(
                                    sp[:, TQ * tqh : TQ * tqh + TQ],
                                    kT[hp0 : hp0 + 64, p, 128 * c : 128 * c + 128],
                                    qT[hp0 : hp0 + 64, p, TQ * tqh : TQ * tqh + TQ],
                                    start=True,
                                    stop=True,
                                )
                            sps.append(sp)
                        ptc = []
                        for hi, (sp, eb_) in enumerate(zip(sps, (ebA, ebB))):
                            pt0 = pt0p.tile([128, 2 * TQ], BF16, tag="pt0")
                            nc.scalar.activation(pt0[:], sp[:], AF.Exp, scale=0.125)
                            s0 = 1023 + 128 * c
                            pt_ = ptp.tile([128, 2 * TQ], BF16, tag="pt")
                            nc.vector.tensor_tensor(
                                pt_[:], pt0[:],
                                eb_[:, s0 - (2 * TQ - 1) : s0 + 1][:, ::-1],
                                ALU.mult,
                            )
                            ptc.append(pt_)
                        pts[c] = ptc
                        if c >= 1:
                            emit_pv(c - 1)
                    emit_pv(7)
                    pend[p] = {"ops": ops, "onum": [[], []], "dd": [], "rep": []}

                # ---------- schedule ----------
                emit_qk_job("q", xq, wqT, qT, bqs, 0)
                emit_qk_job("k", xk, wkT, kT, bks, 0)
                emit_v_job(0)
                emit_v_job(1)
                nc.vector.memset(vp[:, :, :, D : D + 1], 1.0)
                for hh in range(4):  # eb slabs for pairs 0 and 1
                    emit_eb(hh)

                def add_hook(hooks, c, fn):
                    hooks.setdefault(c, []).append(fn)

                for p in range(8):
                    hooks = {}
                    if p + 1 < 8:
                        add_hook(hooks, 0, lambda p=p: emit_eb(2 * p + 2))
                        add_hook(hooks, 0, lambda p=p: emit_eb(2 * p + 3))
                    if p == 0:
                        for j, c in ((2, 0), (3, 1), (4, 2), (5, 3), (6, 4), (7, 5)):
                            add_hook(hooks, c, lambda j=j: emit_v_job(j))
                    if p >= 1:
                        add_hook(hooks, 1, lambda p=p: norm_stage_a(p - 1))
                        add_hook(hooks, 5, lambda p=p: norm_stage_b(p - 1))
                    if p >= 2:
                        add_hook(hooks, 3, lambda p=p: norm_stage_c(p - 2))
                    if p + 1 < 8:
                        add_hook(hooks, 6,
                                 lambda p=p: emit_qk_job("q", xq, wqT, qT, bqs, p + 1))
                        add_hook(hooks, 7,
                                 lambda p=p: emit_qk_job("k", xk, wkT, kT, bks, p + 1))
                    emit_pair(p, hooks)
                norm_stage_c(6)
                norm_stage_a(7)
                norm_stage_b(7)
                norm_stage_c(7)

                # ---------- phase 3: output projection ----------
                wo_tiles = {}
                for co in range(8):
                    wt_ = wmvp.tile([128, 2 * TQ], BF16, tag=f"wmv{co}", name=f"wo{co}")
                    nc.sync.dma_start(out=wt_[:], in_=woT[128 * co : 128 * co + 128, :])
                    wo_tiles[co] = wt_
                for to in range(8):
                    sp = spp.tile([128, 2 * TQ], F32, tag="SP", name=f"y{to}")
                    for fh in range(2):
                        for co in range(8):
                            nc.tensor.matmul(
                                sp[:, TQ * fh : TQ * fh + TQ],
                                oT[:, co, 128 * to : 128 * to + 128],
                                wo_tiles[co][:, TQ * fh : TQ * fh + TQ],
                                start=(co == 0),
                                stop=(co == 7),
                            )
                    yst = ystp.tile([128, 2 * TQ], F32, tag="yst")
                    nc.vector.tensor_tensor(yst[:], sp[:], borep[:], ALU.add)
                    nc.sync.dma_start(
                        out=y_out[128 * to : 128 * to + 128, :], in_=yst[:]
                    )

    _split_multi_waits(nc)
    return nc


_NC_CACHE = None


def _get_nc():
    global _NC_CACHE
    if _NC_CACHE is None:
        _NC_CACHE = _build()
    return _NC_CACHE


def _bf(x):
    return np.ascontiguousarray(np.asarray(x, np.float32).astype(ml_dtypes.bfloat16))


def _prepare_in_maps(
    query, key_, value, Wq, bq, Wk, bk, Wv, bv, Wo, bo, bias_table, offset
):
    query = np.asarray(query, np.float32)
    key_ = np.asarray(key_, np.float32)
    value = np.asarray(value, np.float32)
    shared = {
        "wqT": _bf(np.asarray(Wq, np.float32).T),
        "wkT": _bf(np.asarray(Wk, np.float32).T),
        "wvT": _bf(np.asarray(Wv, np.float32).T),
        "woT": _bf(np.asarray(Wo, np.float32).T),
        "bq2": np.ascontiguousarray(np.asarray(bq, np.float32).reshape(8, 128).T),
        "bk2": np.ascontiguousarray(np.asarray(bk, np.float32).reshape(8, 128).T),
        "bv1": np.ascontiguousarray(np.asarray(bv, np.float32)),
        "bo1": np.ascontiguousarray(np.asarray(bo, np.float32)),
        "offs": np.ascontiguousarray(np.asarray(offset, np.float32)),
    }
    tab = np.asarray(bias_table, np.float32)  # [2T-1, H]
    pad = np.concatenate([tab[0:1], tab, tab[-1:]], axis=0)  # [2T+1, H]
    shared["rtabp"] = np.ascontiguousarray(pad[::-1].T)  # [H, 2T+1]

    in_maps = []
    for b in range(B):
        m = dict(shared)
        m["xqT"] = _bf(query[b].T)
        m["xkT"] = _bf(key_[b].T)
        m["xvT"] = _bf(value[b].T)
        in_maps.append(m)
    return in_maps


def kernel(**inputs):
    in_maps = _prepare_in_maps(
        inputs["query"], inputs["key_"], inputs["value"],
        inputs["Wq"], inputs["bq"], inputs["Wk"], inputs["bk"],
        inputs["Wv"], inputs["bv"], inputs["Wo"], inputs["bo"],
        inputs["bias_table"], inputs["offset"],
    )
    nc = _get_nc()
    res = run_bass_kernel_spmd(nc, in_maps, list(range(B)))
    out = np.stack([res.results[b]["y"] for b in range(B)], axis=0)
    return out.astype(np.float32)
